# revision 8
# baseline (speedup 1.0000x reference)
"""CrossAttention (B=2, N=M=2048, 16 heads x 64) on 8 TRN2 NeuronCores.

Sharding: data-parallel over batch (2) x tensor-parallel over heads (4 per
core). Partial outputs (row-slices of Wo) are summed on host.

Design, tuned against the TRN2 instruction-cost timeline model:
- exp() on ACT is the bottleneck engine (~133us busy): everything else is
  scheduled to hide under it via an explicit filler plan inside the
  attention units.
- QK^T for heads 0,1 runs in fp8e4(e4m3) DoubleRow mode: the d=64
  contraction is folded to [32 partitions, 2 halves], processed at 0.5
  cycles/row (2x the 16-bit rate). Heads 2,3 stay fp16 to keep the
  overall rounding error ~1.3% (gate is 2e-2); the extra PE time hides
  under ACT.
- All 16-bit tensors are fp16 (not bf16) so non-fp8 rounding is
  negligible.
- PV is token-major: out[q, d] = P[ktok, q].T @ Vaug[ktok, d+1]; each
  accumulation matmul moves only 65 columns instead of 512 (PE cost is
  proportional to moving-side columns only; stationary loads are free).
  The softmax denominator rides along as V's ones-column; normalize is a
  per-partition reciprocal+scalar-mul on DVE; a PE transpose returns the
  tile to feature-major for the output projection.
- The last attention unit is emitted as two 512-wide passes so half of
  the tail work (PV chunks 0..3 + output projection of q 1024:1536)
  hides under the second pass's exps; the true epilogue finals use the
  then-idle ACT engine for their PSUM->SBUF copies.
- Inputs are host-relayouted to [128, kt, tokens] so each tensor loads
  with one dma_start (SP issue costs 565ns each); output is fp16 to
  halve the store traffic.
- A short PE warmup keeps the tensor engine's p-state ramp at full
  speed through the DMA-bound prologue.
"""

import sys

if "/opt/trn_rl_repo" not in sys.path:
    sys.path.insert(0, "/opt/trn_rl_repo")

from contextlib import ExitStack

import ml_dtypes
import numpy as np

import concourse.bass as bass
import concourse.mybir as mybir
import concourse.tile as tile
from concourse import bacc
from concourse.bass_utils import run_bass_kernel_spmd
from concourse.masks import make_identity

HEADS = 16
DH = 64
QD = 1024  # query/context feature dim
NN = 2048  # query tokens
MM = 2048  # context tokens
NCORES = 8
HPC = HEADS // (NCORES // 2)  # 4 heads per core
HD = HPC * DH  # 256 inner cols per core
KT = QD // 128  # 8 contraction tiles for projections
TT = MM // 128  # 16 context-token tiles

F32 = mybir.dt.float32
F8 = mybir.dt.float8e4
F16 = mybir.dt.float16

_CACHE = {}


def _build():
    nc = bacc.Bacc("TRN2", target_bir_lowering=False, debug=False)
    xT = nc.declare_dram_parameter("xT", [128, KT, NN], F16, isOutput=False)
    cT = nc.declare_dram_parameter("cT", [128, KT, MM], F16, isOutput=False)
    wq = nc.declare_dram_parameter("wq", [128, KT, HD], F16, isOutput=False)
    wk = nc.declare_dram_parameter("wk", [128, KT, HD], F16, isOutput=False)
    wv = nc.declare_dram_parameter("wv", [128, KT, HD], F16, isOutput=False)
    wo = nc.declare_dram_parameter("wo", [128, 2, QD], F16, isOutput=False)
    out = nc.declare_dram_parameter("out", [QD, NN], F16, isOutput=True)
    with tile.TileContext(nc) as tc:
        _emit(tc, xT, cT, wq, wk, wv, wo, out)
    nc.compile()
    return nc


def _emit(tc, xT, cT, wq, wk, wv, wo, out):
    nc = tc.nc
    Exp = mybir.ActivationFunctionType.Exp
    Copy = mybir.ActivationFunctionType.Copy
    DR = mybir.MatmulPerfMode.DoubleRow

    ctx = ExitStack()
    persist = ctx.enter_context(tc.tile_pool(name="persist", bufs=1))
    xs = persist.tile([128, KT, NN], F16, tag="xs")
    cs = persist.tile([128, KT, MM], F16, tag="cs")
    wqs = persist.tile([128, KT, HD], F16, tag="wqs")
    wks = persist.tile([128, KT, HD], F16, tag="wks")
    wvs = persist.tile([128, KT, HD], F16, tag="wvs")
    wos = persist.tile([128, 2, QD], F16, tag="wos")
    qs = persist.tile([128, 2, NN], F16, tag="qs")  # fp16 q/k (jb1 + fold src)
    ks = persist.tile([128, 2, MM], F16, tag="ks")
    # fp8 folded q/k for DoubleRow QK^T (heads 0,1): [p, dd, half, tok],
    # contraction element d = half*32 + p for head dd.
    qs8 = persist.tile([32, 2, 2, NN], F8, tag="qs8")
    ks8 = persist.tile([32, 2, 2, MM], F8, tag="ks8")
    vs = persist.tile([128, TT, HPC, DH + 1], F16, tag="vs")  # + ones col
    pvs = persist.tile([128, 2, NN], F16, tag="pvs")  # feature-major attnV
    pvn = persist.tile([128, 2, 8, HPC, DH], F16, tag="pvn")  # token-major
    ident = persist.tile([128, 128], F16, tag="ident")
    warm = persist.tile([128, 512], F16, tag="warm")

    qkp = ctx.enter_context(tc.tile_pool(name="qkp", bufs=2, space="PSUM"))
    pvp = ctx.enter_context(tc.tile_pool(name="pvp", bufs=2, space="PSUM"))
    projp = ctx.enter_context(tc.tile_pool(name="projp", bufs=2, space="PSUM"))
    esp = ctx.enter_context(tc.tile_pool(name="esp", bufs=28))
    outp = ctx.enter_context(tc.tile_pool(name="outp", bufs=3))
    nrmp = ctx.enter_context(tc.tile_pool(name="nrmp", bufs=6))

    # ---- DMA issue order = transfer order (single modeled DMA resource).
    # Critical prefix feeds k(jb0,i4=0) then q(jb0,i4=0,1).
    nc.sync.dma_start(wks[:, :, :], wk[:, :, :])
    nc.sync.dma_start(cs[:, :, 0:512], cT[:, :, 0:512])
    nc.sync.dma_start(wqs[:, :, :], wq[:, :, :])
    nc.sync.dma_start(xs[:, :, 0:512], xT[:, :, 0:512])
    nc.sync.dma_start(xs[:, :, 512:1024], xT[:, :, 512:1024])
    nc.sync.dma_start(wvs[:, :, :], wv[:, :, :])
    nc.sync.dma_start(cs[:, :, 512:1024], cT[:, :, 512:1024])
    nc.sync.dma_start(cs[:, :, 1024:1536], cT[:, :, 1024:1536])
    nc.sync.dma_start(cs[:, :, 1536:2048], cT[:, :, 1536:2048])
    nc.sync.dma_start(xs[:, :, 1024:1536], xT[:, :, 1024:1536])
    nc.sync.dma_start(xs[:, :, 1536:2048], xT[:, :, 1536:2048])
    nc.sync.dma_start(wos[:, :, :], wo[:, :, :])

    nc.gpsimd.memset(warm[:, :], 0.25)
    nc.gpsimd.memset(vs[:, :, :, DH:DH + 1], 1.0)
    make_identity(nc, ident[:, :])

    def warmup():
        wp = projp.tile([128, 512], F32, tag="proj", name="wm")
        nc.tensor.matmul(wp[0:64, :], lhsT=warm[:, 0:64], rhs=warm[:, :],
                         start=True, stop=True)

    def qk_chain(dst, dst8, w, src, jb, i4, defer_dd1=False):
        # q/k projection for head-pair jb, token block i4 (512 wide).
        # jb0: fold PSUM directly to fp8 on DVE (dd0 first — it gates the
        # first attention unit); jb1: stage to fp16 SBUF for direct QK.
        ps = projp.tile([128, 512], F32, tag="proj", name="ps")
        for k in range(KT):
            nc.tensor.matmul(
                ps[:, :],
                lhsT=w[:, k, jb * 128:(jb + 1) * 128],
                rhs=src[:, k, i4 * 512:(i4 + 1) * 512],
                start=(k == 0),
                stop=(k == KT - 1),
            )
        c0, c1 = i4 * 512, (i4 + 1) * 512
        if jb == 1:
            nc.vector.tensor_copy(dst[:, 1, c0:c1], ps[:, :])
            return
        dds = (0,) if defer_dd1 else (0, 1)
        for dd in dds:
            for half in range(2):
                nc.vector.tensor_copy(
                    dst8[:, dd, half, c0:c1],
                    ps[dd * 64 + half * 32:dd * 64 + half * 32 + 32, :],
                )
        if defer_dd1:
            # keep an fp16 copy so the dd1 folds can run later from SBUF
            # (the PSUM tile gets recycled)
            nc.vector.tensor_copy(dst[:, 0, c0:c1], ps[:, :])

    def late_fold(dst, dst8, i4):
        # deferred dd=1 folds for a prologue chain, from the fp16 stage
        c0, c1 = i4 * 512, (i4 + 1) * 512
        for half in range(2):
            nc.gpsimd.tensor_copy(
                dst8[:, 1, half, c0:c1],
                dst[64 + half * 32:64 + half * 32 + 32, 0, c0:c1],
            )

    def v_chain(tt):
        ps = projp.tile([128, HPC, DH], F32, tag="proj", name="vp")
        for k in range(KT):
            nc.tensor.matmul(
                ps[:, :, :],
                lhsT=cs[:, k, tt * 128:(tt + 1) * 128],
                rhs=wvs[:, k, :],
                start=(k == 0),
                stop=(k == KT - 1),
            )
        nc.vector.tensor_copy(vs[:, tt, :, 0:DH], ps[:, :, :])

    def qk_mm(qk_slice, h, tt, c0):
        # one [128 ktok, 512 q] logits matmul for head h
        jb, dd = h // 2, h % 2
        if jb == 0:
            nc.tensor.matmul(
                qk_slice,
                lhsT=ks8[:, dd, :, tt * 128:(tt + 1) * 128],
                rhs=qs8[:, dd, :, c0:c0 + 512],
                start=True, stop=True,
                perf_mode=DR,
            )
        else:
            nc.tensor.matmul(
                qk_slice,
                lhsT=ks[dd * 64:(dd + 1) * 64, 1, tt * 128:(tt + 1) * 128],
                rhs=qs[dd * 64:(dd + 1) * 64, 1, c0:c0 + 512],
                start=True, stop=True,
            )

    def attn_unit(ib2, h, fillers=()):
        # QK^T + exp over 16 ktok tiles x 1024 q cols; fillers paced in.
        fillers = list(fillers)
        nfill = len(fillers)
        done = 0
        es = {}
        for tt in range(TT):
            qk = qkp.tile([128, 1024], F32, tag="qk", name="qk")
            for i01 in range(2):
                qk_mm(qk[:, i01 * 512:(i01 + 1) * 512], h, tt,
                      ib2 * 1024 + i01 * 512)
            e = esp.tile([128, 1024], F16, tag="es", name="es")
            nc.scalar.activation(e[:, :], qk[:, :], Exp, scale=0.125)
            es[tt] = (e, 0)
            while done < (nfill * (tt + 1)) // TT:
                fillers.pop(0)()
                done += 1
        while fillers:
            fillers.pop(0)()
        return es

    def attn_half_unit(ib2, h, half, fillers=()):
        # 512-wide pass (chunks 4*half..4*half+3 of block ib2); returns
        # es keyed like attn_unit, with the chunk base recorded.
        fillers = list(fillers)
        nfill = len(fillers)
        done = 0
        es = {}
        for tt in range(TT):
            qk = qkp.tile([128, 512], F32, tag="qk", name="qkh")
            qk_mm(qk[:, :], h, tt, ib2 * 1024 + half * 512)
            e = esp.tile([128, 512], F16, tag="es", name="esh")
            nc.scalar.activation(e[:, :], qk[:, :], Exp, scale=0.125)
            es[tt] = (e, 4 * half)
            while done < (nfill * (tt + 1)) // TT:
                fillers.pop(0)()
                done += 1
        while fillers:
            fillers.pop(0)()
        return es

    def pv_unit(ib2, h, es, c):
        # token-major PV for one 128-q chunk: accumulate over ktok tiles,
        # then normalize by the ones-column into pvn.
        pv = pvp.tile([128, DH + 1], F32, tag="pv", name="pv")
        for tt in range(TT):
            e, cbase = es[tt]
            nc.tensor.matmul(
                pv[:, :],
                lhsT=e[:, (c - cbase) * 128:(c - cbase + 1) * 128],
                rhs=vs[:, tt, h, :],
                start=(tt == 0),
                stop=(tt == TT - 1),
            )
        rc = nrmp.tile([128, 1], F32, tag="rc", name="rc")
        nc.vector.reciprocal(rc[:, :], pv[:, DH:DH + 1])
        nc.vector.tensor_scalar_mul(pvn[:, ib2, c, h, :], pv[:, 0:DH], rc[:, :])

    def tr_unit(ib2, hp, c):
        # transpose one [128 q, 128 head-pair-inner] tile to feature-major
        tp = projp.tile([128, 128], F16, tag="proj", name="tp")
        nc.tensor.transpose(tp[:, :], pvn[:, ib2, c, 2 * hp:2 * hp + 2, :], ident[:, :])
        nc.vector.tensor_copy(pvs[:, hp, ib2 * 1024 + c * 128:ib2 * 1024 + (c + 1) * 128], tp[:, :])

    def final_unit(ib, ob, on_act=False):
        fp = projp.tile([128, 512], F32, tag="proj", name="fp")
        for t2 in range(2):
            nc.tensor.matmul(
                fp[:, :],
                lhsT=wos[:, t2, ob * 128:(ob + 1) * 128],
                rhs=pvs[:, t2, ib * 512:(ib + 1) * 512],
                start=(t2 == 0), stop=(t2 == 1),
            )
        ot = outp.tile([128, 512], F16, tag="ot", name="ot")
        if on_act:
            nc.scalar.activation(ot[:, :], fp[:, :], Copy, scale=1.0)
        else:
            nc.vector.tensor_copy(ot[:, :], fp[:, :])
        nc.sync.dma_start(out[ob * 128:(ob + 1) * 128, ib * 512:(ib + 1) * 512], ot[:, :])

    # ---- prologue: warmups keep the PE p-state ramp alive through the
    # DMA-bound startup; chains in dependency order, dd0 folds first.
    for _ in range(10):
        warmup()
    qk_chain(ks, ks8, wks, cs, 0, 0, defer_dd1=True)
    for t in range(4):
        v_chain(t)
    warmup()
    qk_chain(qs, qs8, wqs, xs, 0, 0, defer_dd1=True)
    warmup()
    warmup()
    qk_chain(qs, qs8, wqs, xs, 0, 1, defer_dd1=True)

    F = lambda f, *a, **k: (lambda: f(*a, **k))
    u00 = [F(late_fold, ks, ks8, 0),
           F(late_fold, qs, qs8, 0),
           F(late_fold, qs, qs8, 1),
           F(qk_chain, ks, ks8, wks, cs, 0, 1)] + \
          [F(v_chain, t) for t in (4, 5)] + \
          [F(qk_chain, ks, ks8, wks, cs, 0, 2)] + \
          [F(v_chain, t) for t in (6, 7, 8)] + \
          [F(qk_chain, ks, ks8, wks, cs, 0, 3)] + \
          [F(v_chain, t) for t in (9, 10, 11, 12, 13, 14, 15)]
    es = attn_unit(0, 0, u00)

    u01 = [F(pv_unit, 0, 0, es, c) for c in range(8)] + \
          [F(qk_chain, ks, ks8, wks, cs, 1, 0),
           F(qk_chain, qs, qs8, wqs, xs, 1, 0),
           F(qk_chain, qs, qs8, wqs, xs, 1, 1)]
    es = attn_unit(0, 1, u01)

    u02 = [F(qk_chain, ks, ks8, wks, cs, 1, 1),
           F(pv_unit, 0, 1, es, 0), F(pv_unit, 0, 1, es, 1),
           F(qk_chain, ks, ks8, wks, cs, 1, 2),
           F(pv_unit, 0, 1, es, 2), F(pv_unit, 0, 1, es, 3),
           F(qk_chain, ks, ks8, wks, cs, 1, 3)] + \
          [F(pv_unit, 0, 1, es, c) for c in range(4, 8)]
    es = attn_unit(0, 2, u02)

    u03 = [F(pv_unit, 0, 2, es, c) for c in range(8)] + \
          [F(tr_unit, 0, 0, c) for c in range(8)] + \
          [F(qk_chain, qs, qs8, wqs, xs, 0, 2),
           F(qk_chain, qs, qs8, wqs, xs, 0, 3)]
    es = attn_unit(0, 3, u03)

    u10 = []
    for c in range(8):
        u10.append(F(pv_unit, 0, 3, es, c))
        u10.append(F(tr_unit, 0, 1, c))
    u10 += [F(final_unit, 0, ob) for ob in range(8)]
    u10 += [F(final_unit, 1, ob) for ob in range(8)]
    es = attn_unit(1, 0, u10)

    u11 = [F(pv_unit, 1, 0, es, c) for c in range(8)] + \
          [F(qk_chain, qs, qs8, wqs, xs, 1, 2),
           F(qk_chain, qs, qs8, wqs, xs, 1, 3)]
    es = attn_unit(1, 1, u11)

    u12 = [F(pv_unit, 1, 1, es, c) for c in range(8)]
    es = attn_unit(1, 2, u12)

    # ---- last unit (1, h3): two 512-wide passes. Pass A's exps hide
    # PV(1,h2) + its transposes; pass B's exps hide pass A's PV chunks,
    # their transposes, and the q 1024:1536 output projection.
    u13 = [F(pv_unit, 1, 2, es, c) for c in range(8)] + \
          [F(tr_unit, 1, 0, c) for c in range(8)]
    esA = attn_half_unit(1, 3, 0, u13)
    ub = []
    for c in range(4):
        ub.append(F(pv_unit, 1, 3, esA, c))
        ub.append(F(tr_unit, 1, 1, c))
    ub += [F(final_unit, 2, ob) for ob in range(8)]
    esB = attn_half_unit(1, 3, 1, ub)

    # ---- epilogue: PV chunks 4..7 + finals for q 1536:2048 (ACT copies)
    for c in range(4, 8):
        pv_unit(1, 3, esB, c)
        tr_unit(1, 1, c)
    for ob in range(8):
        final_unit(3, ob, on_act=True)
    ctx.close()


def _relayout(a, kt):
    # [kt*128, F] -> [128, kt, F]
    f = a.shape[1]
    return np.ascontiguousarray(
        a.reshape(kt, 128, f).transpose(1, 0, 2)
    ).astype(np.float16)


def _inputs_for_core(c, x, context, Wq, Wk, Wv, Wo):
    b, g = c // (NCORES // 2), c % (NCORES // 2)
    sl = slice(g * HD, (g + 1) * HD)
    key = ("xc", b)
    if key not in _CACHE:
        _CACHE[key] = (
            _relayout(np.ascontiguousarray(x[b].T), KT),
            _relayout(np.ascontiguousarray(context[b].T), KT),
        )
    xTb, cTb = _CACHE[key]
    return {
        "xT": xTb,
        "cT": cTb,
        "wq": _relayout(np.ascontiguousarray(Wq[:, sl]), KT),
        "wk": _relayout(np.ascontiguousarray(Wk[:, sl]), KT),
        "wv": _relayout(np.ascontiguousarray(Wv[:, sl]), KT),
        "wo": _relayout(np.ascontiguousarray(Wo[sl, :]), 2),
    }


def kernel(x, context, Wq, Wk, Wv, Wo, bo):
    x = np.asarray(x, np.float32)
    context = np.asarray(context, np.float32)
    if "nc" not in _CACHE:
        _CACHE["nc"] = _build()
    _CACHE.pop(("xc", 0), None)
    _CACHE.pop(("xc", 1), None)
    nc = _CACHE["nc"]
    in_maps = [
        _inputs_for_core(c, x, context, np.asarray(Wq), np.asarray(Wk),
                         np.asarray(Wv), np.asarray(Wo))
        for c in range(NCORES)
    ]
    res = run_bass_kernel_spmd(nc, in_maps, list(range(NCORES))).results
    B = x.shape[0]
    G = NCORES // B
    outp = np.empty((B, NN, QD), np.float32)
    for b in range(B):
        acc = res[b * G]["out"].astype(np.float32)
        for g in range(1, G):
            acc = acc + res[b * G + g]["out"].astype(np.float32)
        outp[b] = acc.T + np.asarray(bo, np.float32)[None, :]
    return outp


# revision 11
# speedup vs baseline: 1.0441x; 1.0441x over previous
"""CrossAttention (B=2, N=M=2048, 16 heads x 64) on 8 TRN2 NeuronCores.

Sharding: data-parallel over batch (2) x tensor-parallel over heads (4 per
core). Partial outputs (row-slices of Wo) are summed on host.

Design, tuned against the TRN2 instruction-cost timeline model:
- exp() on ACT is the bottleneck engine (~133us busy): everything else is
  scheduled to hide under it via an explicit filler plan inside the
  attention units.
- QK^T for heads 0,1 runs in fp8e4(e4m3) DoubleRow mode: the d=64
  contraction is folded to [32 partitions, 2 halves], processed at 0.5
  cycles/row (2x the 16-bit rate). Heads 2,3 stay fp16 to keep the
  overall rounding error ~1.3% (gate is 2e-2); the extra PE time hides
  under ACT.
- All 16-bit tensors are fp16 (not bf16) so non-fp8 rounding is
  negligible.
- PV is token-major: out[q, d] = P[ktok, q].T @ Vaug[ktok, d+1]; each
  accumulation matmul moves only 65 columns instead of 512 (PE cost is
  proportional to moving-side columns only; stationary loads are free).
  The softmax denominator rides along as V's ones-column; normalize is a
  per-partition reciprocal+scalar-mul on DVE; a PE transpose returns the
  tile to feature-major for the output projection.
- The last attention unit is emitted as two 512-wide passes so half of
  the tail work (PV chunks 0..3 + output projection of q 1024:1536)
  hides under the second pass's exps; the true epilogue finals use the
  then-idle ACT engine for their PSUM->SBUF copies.
- Inputs are host-relayouted to [128, kt, tokens] so each tensor loads
  with one dma_start (SP issue costs 565ns each); output is fp16 to
  halve the store traffic.
- A short PE warmup keeps the tensor engine's p-state ramp at full
  speed through the DMA-bound prologue.
"""

import sys

if "/opt/trn_rl_repo" not in sys.path:
    sys.path.insert(0, "/opt/trn_rl_repo")

from contextlib import ExitStack

import ml_dtypes
import numpy as np

import concourse.bass as bass
import concourse.mybir as mybir
import concourse.tile as tile
from concourse import bacc
from concourse.bass_utils import run_bass_kernel_spmd
from concourse.masks import make_identity

HEADS = 16
DH = 64
QD = 1024  # query/context feature dim
NN = 2048  # query tokens
MM = 2048  # context tokens
NCORES = 8
HPC = HEADS // (NCORES // 2)  # 4 heads per core
HD = HPC * DH  # 256 inner cols per core
KT = QD // 128  # 8 contraction tiles for projections
TT = MM // 128  # 16 context-token tiles

F32 = mybir.dt.float32
F8 = mybir.dt.float8e4
F16 = mybir.dt.float16

_CACHE = {}


def _build():
    nc = bacc.Bacc("TRN2", target_bir_lowering=False, debug=False)
    xT = nc.declare_dram_parameter("xT", [128, KT, NN], F16, isOutput=False)
    cT = nc.declare_dram_parameter("cT", [128, KT, MM], F16, isOutput=False)
    wq = nc.declare_dram_parameter("wq", [128, KT, HD], F16, isOutput=False)
    wk = nc.declare_dram_parameter("wk", [128, KT, HD], F16, isOutput=False)
    wv = nc.declare_dram_parameter("wv", [128, KT, HD], F16, isOutput=False)
    wo = nc.declare_dram_parameter("wo", [128, 2, QD], F16, isOutput=False)
    out = nc.declare_dram_parameter("out", [QD, NN], F16, isOutput=True)
    with tile.TileContext(nc) as tc:
        _emit(tc, xT, cT, wq, wk, wv, wo, out)
    nc.compile()
    return nc


def _emit(tc, xT, cT, wq, wk, wv, wo, out):
    nc = tc.nc
    Exp = mybir.ActivationFunctionType.Exp
    Copy = mybir.ActivationFunctionType.Copy
    DR = mybir.MatmulPerfMode.DoubleRow

    ctx = ExitStack()
    persist = ctx.enter_context(tc.tile_pool(name="persist", bufs=1))
    xs = persist.tile([128, KT, NN], F16, tag="xs")
    cs = persist.tile([128, KT, MM], F16, tag="cs")
    wqs = persist.tile([128, KT, HD], F16, tag="wqs")
    wks = persist.tile([128, KT, HD], F16, tag="wks")
    wvs = persist.tile([128, KT, HD], F16, tag="wvs")
    wos = persist.tile([128, 2, QD], F16, tag="wos")
    qs = persist.tile([128, 2, NN], F16, tag="qs")  # fp16 q/k (jb1 + fold src)
    ks = persist.tile([128, 2, MM], F16, tag="ks")
    # fp8 folded q/k for DoubleRow QK^T (heads 0,1): [p, dd, half, tok],
    # contraction element d = half*32 + p for head dd.
    qs8 = persist.tile([32, 2, 2, NN], F8, tag="qs8")
    ks8 = persist.tile([32, 2, 2, MM], F8, tag="ks8")
    vs = persist.tile([128, TT, HPC, DH + 1], F16, tag="vs")  # + ones col
    pvs = persist.tile([128, 2, NN], F16, tag="pvs")  # feature-major attnV
    pvn = persist.tile([128, 2, 8, HPC, DH], F16, tag="pvn")  # token-major
    ident = persist.tile([128, 128], F16, tag="ident")
    warm = persist.tile([128, 512], F16, tag="warm")

    qkp = ctx.enter_context(tc.tile_pool(name="qkp", bufs=2, space="PSUM"))
    pvp = ctx.enter_context(tc.tile_pool(name="pvp", bufs=2, space="PSUM"))
    projp = ctx.enter_context(tc.tile_pool(name="projp", bufs=2, space="PSUM"))
    esp = ctx.enter_context(tc.tile_pool(name="esp", bufs=32))
    outp = ctx.enter_context(tc.tile_pool(name="outp", bufs=3))
    nrmp = ctx.enter_context(tc.tile_pool(name="nrmp", bufs=6))

    # ---- DMA issue order = transfer order (single modeled DMA resource).
    # Critical prefix feeds k(jb0,i4=0) then q(jb0,i4=0,1).
    nc.sync.dma_start(wks[:, :, :], wk[:, :, :])
    nc.sync.dma_start(cs[:, :, 0:512], cT[:, :, 0:512])
    nc.sync.dma_start(wvs[:, :, :], wv[:, :, :])
    nc.sync.dma_start(cs[:, :, 512:1024], cT[:, :, 512:1024])
    nc.sync.dma_start(wqs[:, :, :], wq[:, :, :])
    nc.sync.dma_start(xs[:, :, 0:512], xT[:, :, 0:512])
    nc.sync.dma_start(xs[:, :, 512:1024], xT[:, :, 512:1024])
    nc.sync.dma_start(cs[:, :, 1024:1536], cT[:, :, 1024:1536])
    nc.sync.dma_start(cs[:, :, 1536:2048], cT[:, :, 1536:2048])
    nc.sync.dma_start(xs[:, :, 1024:1536], xT[:, :, 1024:1536])
    nc.sync.dma_start(xs[:, :, 1536:2048], xT[:, :, 1536:2048])
    nc.sync.dma_start(wos[:, :, :], wo[:, :, :])

    nc.gpsimd.memset(warm[:, :], 0.25)
    nc.gpsimd.memset(vs[:, :, :, DH:DH + 1], 1.0)
    make_identity(nc, ident[:, :])

    def warmup():
        wp = projp.tile([128, 512], F32, tag="proj", name="wm")
        nc.tensor.matmul(wp[0:64, :], lhsT=warm[:, 0:64], rhs=warm[:, :],
                         start=True, stop=True)

    def qk_chain(dst, dst8, w, src, jb, i4, defer_dd1=False):
        # q/k projection for head-pair jb, token block i4 (512 wide).
        # jb0: fold PSUM directly to fp8 on DVE (dd0 first — it gates the
        # first attention unit); jb1: stage to fp16 SBUF for direct QK.
        ps = projp.tile([128, 512], F32, tag="proj", name="ps")
        for k in range(KT):
            nc.tensor.matmul(
                ps[:, :],
                lhsT=w[:, k, jb * 128:(jb + 1) * 128],
                rhs=src[:, k, i4 * 512:(i4 + 1) * 512],
                start=(k == 0),
                stop=(k == KT - 1),
            )
        c0, c1 = i4 * 512, (i4 + 1) * 512
        if jb == 1:
            nc.vector.tensor_copy(dst[:, 1, c0:c1], ps[:, :])
            return
        dds = (0,) if defer_dd1 else (0, 1)
        for dd in dds:
            for half in range(2):
                nc.vector.tensor_copy(
                    dst8[:, dd, half, c0:c1],
                    ps[dd * 64 + half * 32:dd * 64 + half * 32 + 32, :],
                )
        if defer_dd1:
            # keep an fp16 copy so the dd1 folds can run later from SBUF
            # (the PSUM tile gets recycled)
            nc.vector.tensor_copy(dst[:, 0, c0:c1], ps[:, :])

    def late_fold(dst, dst8, i4):
        # deferred dd=1 folds for a prologue chain, from the fp16 stage
        c0, c1 = i4 * 512, (i4 + 1) * 512
        for half in range(2):
            nc.gpsimd.tensor_copy(
                dst8[:, 1, half, c0:c1],
                dst[64 + half * 32:64 + half * 32 + 32, 0, c0:c1],
            )

    def v_chain(tt):
        ps = projp.tile([128, HPC, DH], F32, tag="proj", name="vp")
        for k in range(KT):
            nc.tensor.matmul(
                ps[:, :, :],
                lhsT=cs[:, k, tt * 128:(tt + 1) * 128],
                rhs=wvs[:, k, :],
                start=(k == 0),
                stop=(k == KT - 1),
            )
        nc.vector.tensor_copy(vs[:, tt, :, 0:DH], ps[:, :, :])

    def qk_mm(qk_slice, h, tt, c0):
        # one [128 ktok, 512 q] logits matmul for head h
        jb, dd = h // 2, h % 2
        if jb == 0:
            nc.tensor.matmul(
                qk_slice,
                lhsT=ks8[:, dd, :, tt * 128:(tt + 1) * 128],
                rhs=qs8[:, dd, :, c0:c0 + 512],
                start=True, stop=True,
                perf_mode=DR,
            )
        else:
            nc.tensor.matmul(
                qk_slice,
                lhsT=ks[dd * 64:(dd + 1) * 64, 1, tt * 128:(tt + 1) * 128],
                rhs=qs[dd * 64:(dd + 1) * 64, 1, c0:c0 + 512],
                start=True, stop=True,
            )

    def attn_unit(ib2, h, fillers=()):
        # QK^T + exp over 16 ktok tiles x 1024 q cols; fillers paced in.
        fillers = list(fillers)
        nfill = len(fillers)
        done = 0
        es = {}
        for tt in range(TT):
            qk = qkp.tile([128, 1024], F32, tag="qk", name="qk")
            for i01 in range(2):
                qk_mm(qk[:, i01 * 512:(i01 + 1) * 512], h, tt,
                      ib2 * 1024 + i01 * 512)
            e = esp.tile([128, 1024], F16, tag="es", name="es")
            nc.scalar.activation(e[:, :], qk[:, :], Exp, scale=0.125)
            es[tt] = (e, 0)
            while done < (nfill * (tt + 1)) // TT:
                fillers.pop(0)()
                done += 1
        while fillers:
            fillers.pop(0)()
        return es

    def attn_half_unit(ib2, h, half, fillers=()):
        # 512-wide pass (chunks 4*half..4*half+3 of block ib2); returns
        # es keyed like attn_unit, with the chunk base recorded.
        fillers = list(fillers)
        nfill = len(fillers)
        done = 0
        es = {}
        for tt in range(TT):
            qk = qkp.tile([128, 512], F32, tag="qk", name="qkh")
            qk_mm(qk[:, :], h, tt, ib2 * 1024 + half * 512)
            e = esp.tile([128, 512], F16, tag="es", name="esh")
            nc.scalar.activation(e[:, :], qk[:, :], Exp, scale=0.125)
            es[tt] = (e, 4 * half)
            while done < (nfill * (tt + 1)) // TT:
                fillers.pop(0)()
                done += 1
        while fillers:
            fillers.pop(0)()
        return es

    def pv_unit(ib2, h, es, c):
        # token-major PV for one 128-q chunk: accumulate over ktok tiles,
        # then normalize by the ones-column into pvn.
        pv = pvp.tile([128, DH + 1], F32, tag="pv", name="pv")
        for tt in range(TT):
            e, cbase = es[tt]
            nc.tensor.matmul(
                pv[:, :],
                lhsT=e[:, (c - cbase) * 128:(c - cbase + 1) * 128],
                rhs=vs[:, tt, h, :],
                start=(tt == 0),
                stop=(tt == TT - 1),
            )
        rc = nrmp.tile([128, 1], F32, tag="rc", name="rc")
        nc.vector.reciprocal(rc[:, :], pv[:, DH:DH + 1])
        nc.vector.tensor_scalar_mul(pvn[:, ib2, c, h, :], pv[:, 0:DH], rc[:, :])

    def tr_unit(ib2, hp, c):
        # transpose one [128 q, 128 head-pair-inner] tile to feature-major
        tp = projp.tile([128, 128], F16, tag="proj", name="tp")
        nc.tensor.transpose(tp[:, :], pvn[:, ib2, c, 2 * hp:2 * hp + 2, :], ident[:, :])
        nc.vector.tensor_copy(pvs[:, hp, ib2 * 1024 + c * 128:ib2 * 1024 + (c + 1) * 128], tp[:, :])

    def final_unit(ib, ob, on_act=False):
        fp = projp.tile([128, 512], F32, tag="proj", name="fp")
        for t2 in range(2):
            nc.tensor.matmul(
                fp[:, :],
                lhsT=wos[:, t2, ob * 128:(ob + 1) * 128],
                rhs=pvs[:, t2, ib * 512:(ib + 1) * 512],
                start=(t2 == 0), stop=(t2 == 1),
            )
        ot = outp.tile([128, 512], F16, tag="ot", name="ot")
        if on_act:
            nc.scalar.activation(ot[:, :], fp[:, :], Copy, scale=1.0)
        else:
            nc.vector.tensor_copy(ot[:, :], fp[:, :])
        nc.sync.dma_start(out[ob * 128:(ob + 1) * 128, ib * 512:(ib + 1) * 512], ot[:, :])

    # ---- prologue: warmups keep the PE p-state ramp alive through the
    # DMA-bound startup; chains in dependency order, dd0 folds first.
    for _ in range(10):
        warmup()
    qk_chain(ks, ks8, wks, cs, 0, 0, defer_dd1=True)
    for t in range(4):
        v_chain(t)
    warmup()
    qk_chain(qs, qs8, wqs, xs, 0, 0, defer_dd1=True)
    warmup()
    warmup()
    qk_chain(qs, qs8, wqs, xs, 0, 1, defer_dd1=True)

    F = lambda f, *a, **k: (lambda: f(*a, **k))
    u00 = [F(qk_chain, ks, ks8, wks, cs, 0, 1),
           F(late_fold, ks, ks8, 0),
           F(late_fold, qs, qs8, 0),
           F(late_fold, qs, qs8, 1)] + \
          [F(v_chain, t) for t in (4, 5)] + \
          [F(qk_chain, ks, ks8, wks, cs, 0, 2)] + \
          [F(v_chain, t) for t in (6, 7)] + \
          [F(qk_chain, ks, ks8, wks, cs, 0, 3)] + \
          [F(v_chain, t) for t in (8, 9, 10, 11, 12, 13, 14, 15)]
    es = attn_unit(0, 0, u00)

    u01 = [F(pv_unit, 0, 0, es, c) for c in range(8)] + \
          [F(qk_chain, ks, ks8, wks, cs, 1, 0),
           F(qk_chain, qs, qs8, wqs, xs, 1, 0),
           F(qk_chain, qs, qs8, wqs, xs, 1, 1)]
    es = attn_unit(0, 1, u01)

    u02 = [F(qk_chain, ks, ks8, wks, cs, 1, 1),
           F(pv_unit, 0, 1, es, 0), F(pv_unit, 0, 1, es, 1),
           F(qk_chain, ks, ks8, wks, cs, 1, 2),
           F(pv_unit, 0, 1, es, 2), F(pv_unit, 0, 1, es, 3),
           F(qk_chain, ks, ks8, wks, cs, 1, 3)] + \
          [F(pv_unit, 0, 1, es, c) for c in range(4, 8)]
    es = attn_unit(0, 2, u02)

    u03 = [F(pv_unit, 0, 2, es, c) for c in range(8)] + \
          [F(tr_unit, 0, 0, c) for c in range(8)] + \
          [F(qk_chain, qs, qs8, wqs, xs, 0, 2),
           F(qk_chain, qs, qs8, wqs, xs, 0, 3)]
    es = attn_unit(0, 3, u03)

    u10 = []
    for c in range(8):
        u10.append(F(pv_unit, 0, 3, es, c))
        u10.append(F(tr_unit, 0, 1, c))
    u10 += [F(final_unit, 0, ob) for ob in range(8)]
    u10 += [F(final_unit, 1, ob) for ob in range(8)]
    es = attn_unit(1, 0, u10)

    u11 = [F(pv_unit, 1, 0, es, c) for c in range(8)] + \
          [F(qk_chain, qs, qs8, wqs, xs, 1, 2),
           F(qk_chain, qs, qs8, wqs, xs, 1, 3)]
    es = attn_unit(1, 1, u11)

    u12 = [F(pv_unit, 1, 1, es, c) for c in range(8)]
    es = attn_unit(1, 2, u12)

    # ---- last unit (1, h3): two 512-wide passes. Pass A's exps hide
    # PV(1,h2) + its transposes; pass B's exps hide pass A's PV chunks,
    # their transposes, and the q 1024:1536 output projection.
    u13 = [F(pv_unit, 1, 2, es, c) for c in range(8)] + \
          [F(tr_unit, 1, 0, c) for c in range(8)]
    esA = attn_half_unit(1, 3, 0, u13)
    ub = []
    for c in range(4):
        ub.append(F(pv_unit, 1, 3, esA, c))
        ub.append(F(tr_unit, 1, 1, c))
    ub += [F(final_unit, 2, ob) for ob in range(8)]
    esB = attn_half_unit(1, 3, 1, ub)

    # ---- epilogue: PV chunks 4..7 + finals for q 1536:2048 (ACT copies)
    for c in range(4, 8):
        pv_unit(1, 3, esB, c)
        tr_unit(1, 1, c)
    for ob in range(8):
        final_unit(3, ob, on_act=True)
    ctx.close()


def _relayout(a, kt):
    # [kt*128, F] -> [128, kt, F]
    f = a.shape[1]
    return np.ascontiguousarray(
        a.reshape(kt, 128, f).transpose(1, 0, 2)
    ).astype(np.float16)


def _inputs_for_core(c, x, context, Wq, Wk, Wv, Wo):
    b, g = c // (NCORES // 2), c % (NCORES // 2)
    sl = slice(g * HD, (g + 1) * HD)
    key = ("xc", b)
    if key not in _CACHE:
        _CACHE[key] = (
            _relayout(np.ascontiguousarray(x[b].T), KT),
            _relayout(np.ascontiguousarray(context[b].T), KT),
        )
    xTb, cTb = _CACHE[key]
    return {
        "xT": xTb,
        "cT": cTb,
        "wq": _relayout(np.ascontiguousarray(Wq[:, sl]), KT),
        "wk": _relayout(np.ascontiguousarray(Wk[:, sl]), KT),
        "wv": _relayout(np.ascontiguousarray(Wv[:, sl]), KT),
        "wo": _relayout(np.ascontiguousarray(Wo[sl, :]), 2),
    }


def kernel(x, context, Wq, Wk, Wv, Wo, bo):
    x = np.asarray(x, np.float32)
    context = np.asarray(context, np.float32)
    if "nc" not in _CACHE:
        _CACHE["nc"] = _build()
    _CACHE.pop(("xc", 0), None)
    _CACHE.pop(("xc", 1), None)
    nc = _CACHE["nc"]
    in_maps = [
        _inputs_for_core(c, x, context, np.asarray(Wq), np.asarray(Wk),
                         np.asarray(Wv), np.asarray(Wo))
        for c in range(NCORES)
    ]
    res = run_bass_kernel_spmd(nc, in_maps, list(range(NCORES))).results
    B = x.shape[0]
    G = NCORES // B
    outp = np.empty((B, NN, QD), np.float32)
    for b in range(B):
        acc = res[b * G]["out"].astype(np.float32)
        for g in range(1, G):
            acc = acc + res[b * G + g]["out"].astype(np.float32)
        outp[b] = acc.T + np.asarray(bo, np.float32)[None, :]
    return outp


# revision 13
# speedup vs baseline: 1.0589x; 1.0142x over previous
"""CrossAttention (B=2, N=M=2048, 16 heads x 64) on 8 TRN2 NeuronCores.

Sharding: data-parallel over batch (2) x tensor-parallel over heads (4 per
core). Partial outputs (row-slices of Wo) are summed on host.

Design, tuned against the TRN2 instruction-cost timeline model:
- exp() on ACT is the bottleneck engine (~133us busy): everything else is
  scheduled to hide under it via an explicit filler plan inside the
  attention units.
- QK^T for heads 0,1 runs in fp8e4(e4m3) DoubleRow mode: the d=64
  contraction is folded to [32 partitions, 2 halves], processed at 0.5
  cycles/row (2x the 16-bit rate). Heads 2,3 stay fp16 to keep the
  overall rounding error ~1.3% (gate is 2e-2); the extra PE time hides
  under ACT.
- All 16-bit tensors are fp16 (not bf16) so non-fp8 rounding is
  negligible.
- PV is token-major: out[q, d] = P[ktok, q].T @ Vaug[ktok, d+1]; each
  accumulation matmul moves only 65 columns instead of 512 (PE cost is
  proportional to moving-side columns only; stationary loads are free).
  The softmax denominator rides along as V's ones-column; normalize is a
  per-partition reciprocal+scalar-mul on DVE; a PE transpose returns the
  tile to feature-major for the output projection.
- The last attention unit is emitted as two 512-wide passes so half of
  the tail work (PV chunks 0..3 + output projection of q 1024:1536)
  hides under the second pass's exps; the true epilogue finals use the
  then-idle ACT engine for their PSUM->SBUF copies.
- Inputs are host-relayouted to [128, kt, tokens] so each tensor loads
  with one dma_start (SP issue costs 565ns each); output is fp16 to
  halve the store traffic.
- A short PE warmup keeps the tensor engine's p-state ramp at full
  speed through the DMA-bound prologue.
"""

import sys

if "/opt/trn_rl_repo" not in sys.path:
    sys.path.insert(0, "/opt/trn_rl_repo")

from contextlib import ExitStack

import ml_dtypes
import numpy as np

import concourse.bass as bass
import concourse.mybir as mybir
import concourse.tile as tile
from concourse import bacc
from concourse.bass_utils import run_bass_kernel_spmd
from concourse.masks import make_identity

HEADS = 16
DH = 64
QD = 1024  # query/context feature dim
NN = 2048  # query tokens
MM = 2048  # context tokens
NCORES = 8
HPC = HEADS // (NCORES // 2)  # 4 heads per core
HD = HPC * DH  # 256 inner cols per core
KT = QD // 128  # 8 contraction tiles for projections
TT = MM // 128  # 16 context-token tiles

F32 = mybir.dt.float32
F8 = mybir.dt.float8e4
F16 = mybir.dt.float16

_CACHE = {}


def _build():
    nc = bacc.Bacc("TRN2", target_bir_lowering=False, debug=False)
    xT = nc.declare_dram_parameter("xT", [128, KT, NN], F16, isOutput=False)
    cT = nc.declare_dram_parameter("cT", [128, KT, MM], F16, isOutput=False)
    wq = nc.declare_dram_parameter("wq", [128, KT, HD], F16, isOutput=False)
    wk = nc.declare_dram_parameter("wk", [128, KT, HD], F16, isOutput=False)
    wv = nc.declare_dram_parameter("wv", [128, KT, HD], F16, isOutput=False)
    wo = nc.declare_dram_parameter("wo", [128, 2, QD], F16, isOutput=False)
    out = nc.declare_dram_parameter("out", [QD, NN], F16, isOutput=True)
    with tile.TileContext(nc) as tc:
        _emit(tc, xT, cT, wq, wk, wv, wo, out)
    nc.compile()
    return nc


def _emit(tc, xT, cT, wq, wk, wv, wo, out):
    nc = tc.nc
    Exp = mybir.ActivationFunctionType.Exp
    Copy = mybir.ActivationFunctionType.Copy
    DR = mybir.MatmulPerfMode.DoubleRow

    ctx = ExitStack()
    persist = ctx.enter_context(tc.tile_pool(name="persist", bufs=1))
    xs = persist.tile([128, KT, NN], F16, tag="xs")
    cs = persist.tile([128, KT, MM], F16, tag="cs")
    wqs = persist.tile([128, KT, HD], F16, tag="wqs")
    wks = persist.tile([128, KT, HD], F16, tag="wks")
    wvs = persist.tile([128, KT, HD], F16, tag="wvs")
    wos = persist.tile([128, 2, QD], F16, tag="wos")
    qs = persist.tile([128, 2, NN], F16, tag="qs")  # fp16 q/k (jb1 + fold src)
    ks = persist.tile([128, 2, MM], F16, tag="ks")
    # fp8 folded q/k for DoubleRow QK^T (heads 0,1): [p, dd, half, tok],
    # contraction element d = half*32 + p for head dd.
    qs8 = persist.tile([32, 2, 2, NN], F8, tag="qs8")
    ks8 = persist.tile([32, 2, 2, MM], F8, tag="ks8")
    vs = persist.tile([128, TT, HPC, DH + 1], F16, tag="vs")  # + ones col
    pvs = persist.tile([128, 2, NN], F16, tag="pvs")  # feature-major attnV
    pvn = persist.tile([128, 2, 8, HPC, DH], F16, tag="pvn")  # token-major
    ident = persist.tile([128, 128], F16, tag="ident")
    warm = persist.tile([128, 512], F16, tag="warm")

    qkp = ctx.enter_context(tc.tile_pool(name="qkp", bufs=2, space="PSUM"))
    pvp = ctx.enter_context(tc.tile_pool(name="pvp", bufs=2, space="PSUM"))
    projp = ctx.enter_context(tc.tile_pool(name="projp", bufs=2, space="PSUM"))
    esp = ctx.enter_context(tc.tile_pool(name="esp", bufs=32))
    outp = ctx.enter_context(tc.tile_pool(name="outp", bufs=3))
    nrmp = ctx.enter_context(tc.tile_pool(name="nrmp", bufs=6))

    # ---- DMA issue order = transfer order (single modeled DMA resource).
    # Critical prefix feeds k(jb0,i4=0) then q(jb0,i4=0,1).
    nc.sync.dma_start(wks[:, :, :], wk[:, :, :])
    nc.sync.dma_start(cs[:, :, 0:512], cT[:, :, 0:512])
    nc.sync.dma_start(wvs[:, :, :], wv[:, :, :])
    nc.sync.dma_start(wqs[:, :, :], wq[:, :, :])
    nc.sync.dma_start(xs[:, :, 0:512], xT[:, :, 0:512])
    nc.sync.dma_start(xs[:, :, 512:1024], xT[:, :, 512:1024])
    nc.sync.dma_start(cs[:, :, 512:1024], cT[:, :, 512:1024])
    nc.sync.dma_start(cs[:, :, 1024:1536], cT[:, :, 1024:1536])
    nc.sync.dma_start(cs[:, :, 1536:2048], cT[:, :, 1536:2048])
    nc.sync.dma_start(xs[:, :, 1024:1536], xT[:, :, 1024:1536])
    nc.sync.dma_start(xs[:, :, 1536:2048], xT[:, :, 1536:2048])
    nc.sync.dma_start(wos[:, :, :], wo[:, :, :])

    nc.gpsimd.memset(warm[:, :], 0.25)
    nc.gpsimd.memset(vs[:, :, :, DH:DH + 1], 1.0)
    make_identity(nc, ident[:, :])

    def warmup():
        wp = projp.tile([128, 512], F32, tag="proj", name="wm")
        nc.tensor.matmul(wp[0:64, :], lhsT=warm[:, 0:64], rhs=warm[:, :],
                         start=True, stop=True)

    def qk_chain(dst, dst8, w, src, jb, i4, defer_dd1=False):
        # q/k projection for head-pair jb, token block i4 (512 wide).
        # jb0: fold PSUM directly to fp8 on DVE (dd0 first — it gates the
        # first attention unit); jb1: stage to fp16 SBUF for direct QK.
        ps = projp.tile([128, 512], F32, tag="proj", name="ps")
        for k in range(KT):
            nc.tensor.matmul(
                ps[:, :],
                lhsT=w[:, k, jb * 128:(jb + 1) * 128],
                rhs=src[:, k, i4 * 512:(i4 + 1) * 512],
                start=(k == 0),
                stop=(k == KT - 1),
            )
        c0, c1 = i4 * 512, (i4 + 1) * 512
        if jb == 1:
            nc.vector.tensor_copy(dst[:, 1, c0:c1], ps[:, :])
            return
        dds = (0,) if defer_dd1 else (0, 1)
        for dd in dds:
            for half in range(2):
                nc.vector.tensor_copy(
                    dst8[:, dd, half, c0:c1],
                    ps[dd * 64 + half * 32:dd * 64 + half * 32 + 32, :],
                )
        if defer_dd1:
            # keep an fp16 copy so the dd1 folds can run later from SBUF
            # (the PSUM tile gets recycled)
            nc.vector.tensor_copy(dst[:, 0, c0:c1], ps[:, :])

    def late_fold(dst, dst8, i4):
        # deferred dd=1 folds for a prologue chain, from the fp16 stage
        c0, c1 = i4 * 512, (i4 + 1) * 512
        for half in range(2):
            nc.gpsimd.tensor_copy(
                dst8[:, 1, half, c0:c1],
                dst[64 + half * 32:64 + half * 32 + 32, 0, c0:c1],
            )

    def v_chain(tt):
        ps = projp.tile([128, HPC, DH], F32, tag="proj", name="vp")
        for k in range(KT):
            nc.tensor.matmul(
                ps[:, :, :],
                lhsT=cs[:, k, tt * 128:(tt + 1) * 128],
                rhs=wvs[:, k, :],
                start=(k == 0),
                stop=(k == KT - 1),
            )
        nc.vector.tensor_copy(vs[:, tt, :, 0:DH], ps[:, :, :])

    def qk_mm(qk_slice, h, tt, c0):
        # one [128 ktok, 512 q] logits matmul for head h
        jb, dd = h // 2, h % 2
        if jb == 0:
            nc.tensor.matmul(
                qk_slice,
                lhsT=ks8[:, dd, :, tt * 128:(tt + 1) * 128],
                rhs=qs8[:, dd, :, c0:c0 + 512],
                start=True, stop=True,
                perf_mode=DR,
            )
        else:
            nc.tensor.matmul(
                qk_slice,
                lhsT=ks[dd * 64:(dd + 1) * 64, 1, tt * 128:(tt + 1) * 128],
                rhs=qs[dd * 64:(dd + 1) * 64, 1, c0:c0 + 512],
                start=True, stop=True,
            )

    def attn_unit(ib2, h, fillers=()):
        # QK^T + exp over 16 ktok tiles x 1024 q cols; fillers paced in.
        fillers = list(fillers)
        nfill = len(fillers)
        done = 0
        es = {}
        for tt in range(TT):
            qk = qkp.tile([128, 1024], F32, tag="qk", name="qk")
            for i01 in range(2):
                qk_mm(qk[:, i01 * 512:(i01 + 1) * 512], h, tt,
                      ib2 * 1024 + i01 * 512)
            e = esp.tile([128, 1024], F16, tag="es", name="es")
            nc.scalar.activation(e[:, :], qk[:, :], Exp, scale=0.125)
            es[tt] = (e, 0)
            while done < (nfill * (tt + 1)) // TT:
                fillers.pop(0)()
                done += 1
        while fillers:
            fillers.pop(0)()
        return es

    def attn_half_unit(ib2, h, half, fillers=()):
        # 512-wide pass (chunks 4*half..4*half+3 of block ib2); returns
        # es keyed like attn_unit, with the chunk base recorded.
        fillers = list(fillers)
        nfill = len(fillers)
        done = 0
        es = {}
        for tt in range(TT):
            qk = qkp.tile([128, 512], F32, tag="qk", name="qkh")
            qk_mm(qk[:, :], h, tt, ib2 * 1024 + half * 512)
            e = esp.tile([128, 512], F16, tag="es", name="esh")
            nc.scalar.activation(e[:, :], qk[:, :], Exp, scale=0.125)
            es[tt] = (e, 4 * half)
            while done < (nfill * (tt + 1)) // TT:
                fillers.pop(0)()
                done += 1
        while fillers:
            fillers.pop(0)()
        return es

    def pv_unit(ib2, h, es, c):
        # token-major PV for one 128-q chunk: accumulate over ktok tiles,
        # then normalize by the ones-column into pvn.
        pv = pvp.tile([128, DH + 1], F32, tag="pv", name="pv")
        for tt in range(TT):
            e, cbase = es[tt]
            nc.tensor.matmul(
                pv[:, :],
                lhsT=e[:, (c - cbase) * 128:(c - cbase + 1) * 128],
                rhs=vs[:, tt, h, :],
                start=(tt == 0),
                stop=(tt == TT - 1),
            )
        rc = nrmp.tile([128, 1], F32, tag="rc", name="rc")
        nc.vector.reciprocal(rc[:, :], pv[:, DH:DH + 1])
        nc.vector.tensor_scalar_mul(pvn[:, ib2, c, h, :], pv[:, 0:DH], rc[:, :])

    def tr_unit(ib2, hp, c):
        # transpose one [128 q, 128 head-pair-inner] tile to feature-major
        tp = projp.tile([128, 128], F16, tag="proj", name="tp")
        nc.tensor.transpose(tp[:, :], pvn[:, ib2, c, 2 * hp:2 * hp + 2, :], ident[:, :])
        nc.vector.tensor_copy(pvs[:, hp, ib2 * 1024 + c * 128:ib2 * 1024 + (c + 1) * 128], tp[:, :])

    def final_unit(ib, ob, on_act=False):
        fp = projp.tile([128, 512], F32, tag="proj", name="fp")
        for t2 in range(2):
            nc.tensor.matmul(
                fp[:, :],
                lhsT=wos[:, t2, ob * 128:(ob + 1) * 128],
                rhs=pvs[:, t2, ib * 512:(ib + 1) * 512],
                start=(t2 == 0), stop=(t2 == 1),
            )
        ot = outp.tile([128, 512], F16, tag="ot", name="ot")
        if on_act:
            nc.scalar.activation(ot[:, :], fp[:, :], Copy, scale=1.0)
        else:
            nc.vector.tensor_copy(ot[:, :], fp[:, :])
        nc.sync.dma_start(out[ob * 128:(ob + 1) * 128, ib * 512:(ib + 1) * 512], ot[:, :])

    # ---- prologue: warmups keep the PE p-state ramp alive through the
    # DMA-bound startup; chains in dependency order, dd0 folds first.
    for _ in range(10):
        warmup()
    qk_chain(ks, ks8, wks, cs, 0, 0, defer_dd1=True)
    for t in range(4):
        v_chain(t)
    warmup()
    qk_chain(qs, qs8, wqs, xs, 0, 0, defer_dd1=True)
    warmup()
    warmup()
    qk_chain(qs, qs8, wqs, xs, 0, 1, defer_dd1=True)
    qk_chain(ks, ks8, wks, cs, 0, 1)

    F = lambda f, *a, **k: (lambda: f(*a, **k))
    u00 = [F(qk_chain, ks, ks8, wks, cs, 0, 2),
           F(late_fold, ks, ks8, 0),
           F(late_fold, qs, qs8, 0),
           F(late_fold, qs, qs8, 1),
           F(qk_chain, ks, ks8, wks, cs, 0, 3)] + \
          [F(v_chain, t) for t in (4, 5, 6, 7, 8, 9, 10, 11, 12, 13, 14, 15)]
    es = attn_unit(0, 0, u00)

    u01 = [F(pv_unit, 0, 0, es, c) for c in range(8)] + \
          [F(qk_chain, ks, ks8, wks, cs, 1, 0),
           F(qk_chain, qs, qs8, wqs, xs, 1, 0),
           F(qk_chain, qs, qs8, wqs, xs, 1, 1)]
    es = attn_unit(0, 1, u01)

    u02 = [F(qk_chain, ks, ks8, wks, cs, 1, 1),
           F(pv_unit, 0, 1, es, 0), F(pv_unit, 0, 1, es, 1),
           F(qk_chain, ks, ks8, wks, cs, 1, 2),
           F(pv_unit, 0, 1, es, 2), F(pv_unit, 0, 1, es, 3),
           F(qk_chain, ks, ks8, wks, cs, 1, 3)] + \
          [F(pv_unit, 0, 1, es, c) for c in range(4, 8)]
    es = attn_unit(0, 2, u02)

    u03 = [F(pv_unit, 0, 2, es, c) for c in range(8)] + \
          [F(tr_unit, 0, 0, c) for c in range(8)] + \
          [F(qk_chain, qs, qs8, wqs, xs, 0, 2),
           F(qk_chain, qs, qs8, wqs, xs, 0, 3)]
    es = attn_unit(0, 3, u03)

    u10 = []
    for c in range(8):
        u10.append(F(pv_unit, 0, 3, es, c))
        u10.append(F(tr_unit, 0, 1, c))
    u10 += [F(final_unit, 0, ob) for ob in range(8)]
    u10 += [F(final_unit, 1, ob) for ob in range(8)]
    es = attn_unit(1, 0, u10)

    u11 = [F(pv_unit, 1, 0, es, c) for c in range(8)] + \
          [F(qk_chain, qs, qs8, wqs, xs, 1, 2),
           F(qk_chain, qs, qs8, wqs, xs, 1, 3)]
    es = attn_unit(1, 1, u11)

    u12 = [F(pv_unit, 1, 1, es, c) for c in range(8)]
    es = attn_unit(1, 2, u12)

    # ---- last unit (1, h3): two 512-wide passes. Pass A's exps hide
    # PV(1,h2) + its transposes; pass B's exps hide pass A's PV chunks,
    # their transposes, and the q 1024:1536 output projection.
    u13 = [F(pv_unit, 1, 2, es, c) for c in range(8)] + \
          [F(tr_unit, 1, 0, c) for c in range(8)]
    esA = attn_half_unit(1, 3, 0, u13)
    ub = []
    for c in range(4):
        ub.append(F(pv_unit, 1, 3, esA, c))
        ub.append(F(tr_unit, 1, 1, c))
    ub += [F(final_unit, 2, ob) for ob in range(8)]
    esB = attn_half_unit(1, 3, 1, ub)

    # ---- epilogue: PV chunks 4..7 + finals for q 1536:2048 (ACT copies)
    for c in range(4, 8):
        pv_unit(1, 3, esB, c)
        tr_unit(1, 1, c)
    for ob in range(8):
        final_unit(3, ob, on_act=True)
    ctx.close()


def _relayout(a, kt):
    # [kt*128, F] -> [128, kt, F]
    f = a.shape[1]
    return np.ascontiguousarray(
        a.reshape(kt, 128, f).transpose(1, 0, 2)
    ).astype(np.float16)


def _inputs_for_core(c, x, context, Wq, Wk, Wv, Wo):
    b, g = c // (NCORES // 2), c % (NCORES // 2)
    sl = slice(g * HD, (g + 1) * HD)
    key = ("xc", b)
    if key not in _CACHE:
        _CACHE[key] = (
            _relayout(np.ascontiguousarray(x[b].T), KT),
            _relayout(np.ascontiguousarray(context[b].T), KT),
        )
    xTb, cTb = _CACHE[key]
    return {
        "xT": xTb,
        "cT": cTb,
        "wq": _relayout(np.ascontiguousarray(Wq[:, sl]), KT),
        "wk": _relayout(np.ascontiguousarray(Wk[:, sl]), KT),
        "wv": _relayout(np.ascontiguousarray(Wv[:, sl]), KT),
        "wo": _relayout(np.ascontiguousarray(Wo[sl, :]), 2),
    }


def kernel(x, context, Wq, Wk, Wv, Wo, bo):
    x = np.asarray(x, np.float32)
    context = np.asarray(context, np.float32)
    if "nc" not in _CACHE:
        _CACHE["nc"] = _build()
    _CACHE.pop(("xc", 0), None)
    _CACHE.pop(("xc", 1), None)
    nc = _CACHE["nc"]
    in_maps = [
        _inputs_for_core(c, x, context, np.asarray(Wq), np.asarray(Wk),
                         np.asarray(Wv), np.asarray(Wo))
        for c in range(NCORES)
    ]
    res = run_bass_kernel_spmd(nc, in_maps, list(range(NCORES))).results
    B = x.shape[0]
    G = NCORES // B
    outp = np.empty((B, NN, QD), np.float32)
    for b in range(B):
        acc = res[b * G]["out"].astype(np.float32)
        for g in range(1, G):
            acc = acc + res[b * G + g]["out"].astype(np.float32)
        outp[b] = acc.T + np.asarray(bo, np.float32)[None, :]
    return outp
